# revision 1
# baseline (speedup 1.0000x reference)
# Trainium2 Bass kernel for nn_DifferentiableProcessor (dense_cnn).
#
# Math restructure: with separable 15-tap gaussian blur B,
#   x5 = (1+e)x4 - e*B(x4);  x6 = s*B(x5) + (1-s)*x5
#   => x6 = a*x4 + b*B(x4) + c*B2(x4),  B2 = B∘B (29-tap per axis; edge-exact
#      coefficients taken from the true matrix square of the truncated operator)
#   a=(1-s)(1+e), b=s(1+e)-(1-s)e, c=-s*e
#
# Sharding: 8 cores, each gets 1.5 of the 12 BxC image planes (one full plane
# + half of a shared plane, 14-row halo at the split), 12 output row-tiles of
# 128 rows per core.
#
# All-f32, transpose-free dataflow ("stationary swap"): both conv directions
# run on the tensor engine with the image tile as the STATIONARY operand and
# small banded coefficient matrices as the MOVING operand, so the vertical
# conv emits column-major tiles and the horizontal conv brings them back to
# row-major, accumulating a*x4 + b*Bv/h + c*B2v/h in PSUM directly.
import numpy as np

_CACHE = {}
LAST = None  # last BassKernelResults (exec_time_ns when BASS_TRACE=1)

KS = 15
PAD = 7
H = 1024
W = 1024
B_, C_ = 4, 3
NCORES = 8
TPC = 12          # output tiles per core
NBLK = 14         # x4 slab blocks per core (9 F-section + 5 G-section)
WP = W + 28       # padded slab block width (14 zero cols each side)
NJ = 9            # column tiles for vT (8 full + 1 mini of 28)


def _gauss1d():
    g = (np.arange(KS) - KS // 2).astype(np.float32)
    g = np.exp(-(g * g) / np.float32(2.0 * 3.0 * 3.0)).astype(np.float32)
    return (g / g.sum()).astype(np.float32)


def _conv_op(n):
    g = _gauss1d().astype(np.float64)
    Gm = np.zeros((n, n))
    for r in range(n):
        lo = max(0, r - PAD)
        hi = min(n, r + PAD + 1)
        Gm[r, lo:hi] = g[lo - r + PAD:hi - r + PAD]
    return Gm


def _section_layout(core):
    full = [0, 2, 3, 5, 6, 8, 9, 11][core]
    shared = [1, 1, 4, 4, 7, 7, 10, 10][core]
    top = core % 2 == 0
    g_off = 0 if top else 512
    tiles = [(full, 128 * t) for t in range(8)] + \
            [(shared, g_off + 128 * t) for t in range(4)]
    blocks = [(full, 128 * b - 14) for b in range(9)] + \
             [(shared, g_off + 128 * b - 14) for b in range(5)]
    return tiles, blocks


def _build_host_data(inputs):
    x = np.asarray(inputs["x"], np.float32)
    gains = np.asarray(inputs["gains"], np.float32)
    sc = {k: float(np.asarray(inputs[k], np.float32)) for k in
          ["gamma", "shadow_boost", "highlight_reduce", "brightness", "contrast",
           "enhance_amount", "softness", "intensity", "rotation", "hardness"]}
    e, s = sc["enhance_amount"], sc["softness"]
    a_sc = (1.0 - s) * (1.0 + e)
    b_sc = s * (1.0 + e) - (1.0 - s) * e
    c_sc = -s * e

    G2_64 = _conv_op(H) @ _conv_op(H)
    g15 = _gauss1d().astype(np.float64)

    def g15v(k):
        return g15[int(k)] if 0 <= k < KS else 0.0

    # ---- v-stage moving bands (toeplitz 15-tap part; G2 part is per-core)
    t15A = np.zeros((128, 128))
    t15B = np.zeros((28, 128))
    for p in range(128):
        for r in range(128):
            t15A[p, r] = g15v(p - r - 7)
    for p in range(28):
        for r in range(128):
            t15B[p, r] = g15v(121 + p - r)

    # ---- H-stage bands rh15/rh29 [*, NJ*256]; slice J serves lhsT = vT tile J
    # (source cols j = 128J-14+p). Mid-J slices cover j'-blocks [J-1 | J] at
    # col offsets [0:128 | 128:256]; J=0 covers block 0 at [0:128]; J=8 covers
    # block 7 at [0:128].
    rh15 = np.zeros((128, NJ * 256))
    rh29 = np.zeros((128, NJ * 256))
    for J in range(NJ):
        npart = 128 if J < 8 else 28
        if J == 0:
            blks = [(0, 0), (1, 128)]
        elif J == 8:
            blks = [(6, 0), (7, 128)]
        else:
            blks = [(J - 1, 0), (J, 128)]
        for p in range(npart):
            j = 128 * J - 14 + p
            for (jb, co) in blks:
                for n in range(128):
                    jp = 128 * jb + n
                    rh15[p, J * 256 + co + n] = b_sc * g15v(jp - j + 7)
                    if 0 <= j < W and abs(jp - j) <= 2 * PAD:
                        rh29[p, J * 256 + co + n] = c_sc * G2_64[jp, j]

    # ---- a*x4 shifted diagonals
    aiA = np.zeros((128, 128), np.float32)
    aiB = np.zeros((14, 128), np.float32)
    for m in range(114):
        aiA[m + 14, m] = a_sc
    for p in range(14):
        aiB[p, 114 + p] = a_sc

    # ---- phase-1 scalar folds
    gamma = sc["gamma"]
    sb, hr = sc["shadow_boost"], sc["highlight_reduce"]
    br, ct = sc["brightness"], sc["contrast"]
    q1 = 0.5 * (sb - hr)
    q2 = -0.5 * (sb + hr)
    beta = 0.5 - 0.5 * ct + br
    lo0 = max(0.0, min(beta, ct + beta))
    hi0 = min(1.0, max(beta, ct + beta))
    cbm0 = ct * q1 + beta
    inten = sc["intensity"]

    # ---- gradient mask vectors (w = (1-i/2) + (i/2)*tanh(0.5*h*grid_rot))
    th = sc["rotation"] * np.pi / 180.0
    ys = np.linspace(-1.0, 1.0, H, dtype=np.float32).astype(np.float64)
    xs = np.linspace(-1.0, 1.0, W, dtype=np.float32).astype(np.float64)
    colv = np.broadcast_to(0.5 * sc["hardness"] * np.cos(th) * xs,
                           (128, W)).astype(np.float32).copy()
    rowmul = 0.5 * sc["hardness"] * np.sin(th)

    per_core = []
    for core in range(NCORES):
        tiles, blocks = _section_layout(core)
        xin = np.zeros((NBLK * 128, WP), np.float32)
        vecs = np.zeros((128, 5 * NBLK), np.float32)
        for b, (pl, gr0) in enumerate(blocks):
            rows = gr0 + np.arange(128)
            valid = (rows >= 0) & (rows < H)
            src = np.clip(rows, 0, H - 1)
            xin[b * 128:(b + 1) * 128, 14:W + 14] = x[pl // C_, pl % C_][src] * valid[:, None]
            m = valid.astype(np.float32)
            vecs[:, 0 * NBLK + b] = gains[pl % C_]
            vecs[:, 1 * NBLK + b] = ct * m
            vecs[:, 2 * NBLK + b] = cbm0 * m
            vecs[:, 3 * NBLK + b] = lo0 * m
            vecs[:, 4 * NBLK + b] = hi0 * m
        rvA = np.zeros((128, TPC * 256))
        rvB = np.zeros((28, TPC * 256))
        rowv = np.zeros((128, TPC), np.float32)
        for t, (pl, T) in enumerate(tiles):
            rvA[:, t * 256:t * 256 + 128] = t15A
            rvB[:, t * 256:t * 256 + 128] = t15B
            for p in range(128):
                ri = T - 14 + p
                if 0 <= ri < H:
                    rvA[p, t * 256 + 128:(t + 1) * 256] = G2_64[T:T + 128, ri]
            for p in range(28):
                ri = T + 114 + p
                if 0 <= ri < H:
                    rvB[p, t * 256 + 128:(t + 1) * 256] = G2_64[T:T + 128, ri]
            rowv[:, t] = (rowmul * ys[T:T + 128]).astype(np.float32)
        per_core.append(dict(
            xin=xin, vecs=vecs, rva=rvA.astype(np.float32),
            rvb=rvB.astype(np.float32), rowv=rowv))

    shared = dict(rh15=rh15.astype(np.float32), rh29=rh29.astype(np.float32),
                  aia=aiA, aib=aiB, colv=colv)
    scalars = dict(gamma=gamma, q2=q2, inten=inten)
    return per_core, shared, scalars


def _build_program(scalars):
    import sys
    if '/opt/trn_rl_repo' not in sys.path:
        sys.path.insert(0, '/opt/trn_rl_repo')
    import concourse.bacc as bacc
    import concourse.mybir as mybir
    from concourse.tile import TileContext
    from concourse.alu_op_type import AluOpType
    A = mybir.ActivationFunctionType
    F32 = mybir.dt.float32
    R32 = mybir.dt.float32r
    rc = lambda ap: ap.bitcast(R32)

    nc = bacc.Bacc()

    def reg_const(v):
        t = nc.alloc_sbuf_tensor(f"constu-f32-{v}", [128, 1], F32)
        nc.gpsimd.memset(t.ap(), v)
        nc.const_aps.aps[(F32, v)] = t.ap()

    for v in (1e-30, -2.5):
        if (F32, v) not in nc.const_aps.aps:
            reg_const(v)
    nc.all_engine_barrier()

    d_xin = nc.dram_tensor("xin", [NBLK * 128, WP], R32, kind="ExternalInput")
    d_vecs = nc.dram_tensor("vecs", [128, 5 * NBLK], F32, kind="ExternalInput")
    d_rva = nc.dram_tensor("rva", [128, TPC * 256], R32, kind="ExternalInput")
    d_rvb = nc.dram_tensor("rvb", [28, TPC * 256], R32, kind="ExternalInput")
    d_rh15 = nc.dram_tensor("rh15", [128, NJ * 256], R32, kind="ExternalInput")
    d_rh29 = nc.dram_tensor("rh29", [128, NJ * 256], R32, kind="ExternalInput")
    d_aia = nc.dram_tensor("aia", [128, 128], R32, kind="ExternalInput")
    d_aib = nc.dram_tensor("aib", [14, 128], R32, kind="ExternalInput")
    d_colv = nc.dram_tensor("colv", [128, W], F32, kind="ExternalInput")
    d_rowv = nc.dram_tensor("rowv", [128, TPC], F32, kind="ExternalInput")
    d_out = nc.dram_tensor("out", [TPC * 128, W], F32, kind="ExternalOutput")

    gamma, q2, inten = scalars["gamma"], scalars["q2"], scalars["inten"]

    with TileContext(nc) as tc:
        with tc.tile_pool(name="const", bufs=1) as cp, \
             tc.tile_pool(name="slab", bufs=1) as sp, \
             tc.tile_pool(name="work", bufs=4) as wp, \
             tc.tile_pool(name="vt", bufs=2) as vp, \
             tc.tile_pool(name="outp", bufs=4) as op, \
             tc.tile_pool(name="psv", bufs=4, space="PSUM") as psv, \
             tc.tile_pool(name="psp", bufs=2, space="PSUM") as psp:

            def load_const(dt, shape, tag, dtype=F32):
                t = cp.tile(shape, dtype, tag=tag)
                nc.sync.dma_start(out=t[:shape[0]], in_=dt[:])
                return t

            vecs = load_const(d_vecs, [128, 5 * NBLK], "c_vecs")
            rva = load_const(d_rva, [128, TPC * 256], "c_rva", R32)
            rvb = load_const(d_rvb, [28, TPC * 256], "c_rvb", R32)
            rh15 = load_const(d_rh15, [128, NJ * 256], "c_rh15", R32)
            rh29 = load_const(d_rh29, [128, NJ * 256], "c_rh29", R32)
            aia = load_const(d_aia, [128, 128], "c_aia", R32)
            aib = load_const(d_aib, [14, 128], "c_aib", R32)
            colv = load_const(d_colv, [128, W], "c_colv")
            rowv = load_const(d_rowv, [128, TPC], "c_rowv")

            blks = []
            for b in range(NBLK):
                bt = sp.tile([128, WP], R32, tag=f"x4b{b}")
                nc.sync.dma_start(out=bt[:], in_=d_xin[b * 128:(b + 1) * 128, :])
                blks.append(bt)

            # ---- phase 1a: u = (gain*x)^gamma via Ln then in-place Exp.
            # Batched per activation function with hard fences so the ACT
            # table is loaded once per function batch, not per interleave.
            for b in range(NBLK):
                nc.scalar.activation(blks[b][:, 14:W + 14],
                                     blks[b][:, 14:W + 14].bitcast(F32), A.Ln,
                                     bias=1e-30, scale=vecs[:, b:b + 1])
            tc.strict_bb_all_engine_barrier()
            for b in range(NBLK):
                nc.scalar.activation(blks[b][:, 14:W + 14],
                                     blks[b][:, 14:W + 14].bitcast(F32), A.Exp,
                                     bias=0.0, scale=gamma)

            # Hard scheduling fence: keep every tanh-set activation after
            # all Ln/Exp-set activations (2 ACT table loads total instead of
            # one per interleave).
            tc.strict_bb_all_engine_barrier()
            colv_live = colv

            # ---- phase 1b: shadows/highlights + contrast + clamp (tanh set)
            for b in range(NBLK):
                uf = blks[b][:, 14:W + 14].bitcast(F32)
                h2 = wp.tile([128, W], F32, tag="p1")
                nc.scalar.activation(h2[:], uf, A.Tanh, bias=-2.5, scale=5.0)
                z = wp.tile([128, W], F32, tag="p1")
                nc.vector.scalar_tensor_tensor(z[:], h2[:], q2, uf,
                                               AluOpType.mult, AluOpType.add)
                x4a = wp.tile([128, W], F32, tag="p1")
                nc.vector.tensor_scalar(x4a[:], z[:],
                                        vecs[:, NBLK + b:NBLK + b + 1],
                                        vecs[:, 2 * NBLK + b:2 * NBLK + b + 1],
                                        AluOpType.mult, AluOpType.add)
                nc.vector.tensor_scalar(blks[b][:, 14:W + 14], x4a[:],
                                        vecs[:, 3 * NBLK + b:3 * NBLK + b + 1],
                                        vecs[:, 4 * NBLK + b:4 * NBLK + b + 1],
                                        AluOpType.max, AluOpType.min)

            # ---- phase 2 + 3, software-pipelined so the PE never waits
            # for the PSUM->SBUF copies of the current tile (v-stage of tile
            # t+1 is issued before the H-stage of tile t).
            secblk = [0] * 8 + [9] * 4
            vts = [None] * TPC

            def vstage(t):
                bA = secblk[t] + (t if t < 8 else t - 8)
                blkA, blkB = blks[bA], blks[bA + 1]
                vt_sb = vp.tile([128, NJ * 256], R32, tag="vt")
                vts[t] = vt_sb
                for J in range(NJ):
                    npart = 128 if J < 8 else 28
                    csl = slice(128 * J, 128 * J + npart)
                    pv = psv.tile([128, 256], F32, tag="pv")
                    nc.tensor.matmul(pv[:npart], lhsT=rc(blkA[:, csl]),
                                     rhs=rc(rva[:, t * 256:(t + 1) * 256]),
                                     start=True, stop=False)
                    nc.tensor.matmul(pv[:npart], lhsT=rc(blkB[0:28, csl]),
                                     rhs=rc(rvb[0:28, t * 256:(t + 1) * 256]),
                                     start=False, stop=True)
                    if J % 2 == 0:
                        nc.vector.tensor_copy(vt_sb[:npart, J * 256:(J + 1) * 256],
                                              pv[:npart])
                    else:
                        nc.scalar.copy(vt_sb[:npart, J * 256:(J + 1) * 256],
                                       pv[:npart])

            def hstage(t):
                bA = secblk[t] + (t if t < 8 else t - 8)
                blkA, blkB = blks[bA], blks[bA + 1]
                vt_sb = vts[t]
                pre = psp.tile([128, W], F32, tag="pre")
                for nh in range(2):
                    nsl = slice(nh * 512, (nh + 1) * 512)
                    nc.tensor.matmul(pre[:, nsl], lhsT=rc(aia[:]),
                                     rhs=rc(blkA[:, 14 + nh * 512:14 + (nh + 1) * 512]),
                                     start=True, stop=False,
                                     skip_group_check=True)
                    nc.tensor.matmul(pre[:, nsl], lhsT=rc(aib[0:14]),
                                     rhs=rc(blkB[0:14, 14 + nh * 512:14 + (nh + 1) * 512]),
                                     start=False, stop=False,
                                     skip_group_check=True)
                for J in range(NJ):
                    npart = 128 if J < 8 else 28
                    if J == 0:
                        osl = slice(0, 256)
                    elif J == 8:
                        osl = slice(768, 1024)
                    else:
                        osl = slice((J - 1) * 128, (J + 1) * 128)
                    ncol = 256
                    nc.tensor.matmul(pre[:, osl],
                                     lhsT=rc(vt_sb[:npart, J * 256:J * 256 + 128]),
                                     rhs=rc(rh15[:npart, J * 256:J * 256 + ncol]),
                                     start=False, stop=False,
                                     skip_group_check=True)
                    nc.tensor.matmul(pre[:, osl],
                                     lhsT=rc(vt_sb[:npart, J * 256 + 128:(J + 1) * 256]),
                                     rhs=rc(rh29[:npart, J * 256:J * 256 + ncol]),
                                     start=False, stop=(J == NJ - 1),
                                     skip_group_check=True)
                tv = op.tile([128, W], F32, tag="p3")
                nc.scalar.activation(tv[:], colv_live[:], A.Tanh,
                                     bias=rowv[:, t:t + 1], scale=1.0)
                wv = op.tile([128, W], F32, tag="p3")
                nc.vector.tensor_scalar(wv[:], tv[:], 0.5 * inten,
                                        1.0 - 0.5 * inten,
                                        AluOpType.mult, AluOpType.add)
                om = op.tile([128, W], F32, tag="p3")
                nc.vector.tensor_tensor(om[:], wv[:], pre[:], AluOpType.mult)
                oc = op.tile([128, W], F32, tag="p3")
                nc.vector.tensor_scalar(oc[:], om[:], 0.0, 1.0,
                                        AluOpType.max, AluOpType.min)
                nc.sync.dma_start(out=d_out[t * 128:(t + 1) * 128, :], in_=oc[:])

            for t in range(TPC + 1):
                if t < TPC:
                    vstage(t)
                if t >= 1:
                    hstage(t - 1)

    nc.finalize()
    return nc


def kernel(**inputs):
    import sys
    if '/opt/trn_rl_repo' not in sys.path:
        sys.path.insert(0, '/opt/trn_rl_repo')
    from concourse.bass_utils import run_bass_kernel_spmd

    per_core, shared, scalars = _build_host_data(inputs)
    key = tuple(sorted(scalars.items()))
    if key not in _CACHE:
        _CACHE[key] = _build_program(scalars)
    nc = _CACHE[key]

    in_maps = [dict(shared, **per_core[c]) for c in range(NCORES)]
    res = run_bass_kernel_spmd(nc, in_maps, core_ids=list(range(NCORES)))
    global LAST
    LAST = res

    x = np.asarray(inputs["x"], np.float32)
    out = np.empty_like(x)
    for core in range(NCORES):
        tiles, _ = _section_layout(core)
        o = res.results[core]["out"]
        for t, (pl, T) in enumerate(tiles):
            out[pl // C_, pl % C_, T:T + 128, :] = o[t * 128:(t + 1) * 128, :]
    return out

